# revision 1
# baseline (speedup 1.0000x reference)
"""DiffusionStep kernel v2: ap_gather gather + GPSIMD scatter_add scatter.

Node map for scatter: n = q*256 + 2*ch + s  (q = src>>8 in [0,391),
ch = (src&255)>>1 selects the partition, s = src&1 the d-slot).
acc_sb [128, 391, 2] bf16 accumulates per chunk, drained to f32 each chunk.
"""
import sys

sys.path.insert(0, "/opt/trn_rl_repo")

import numpy as np

N = 100000
E = 6400000
NCORES = 8
EC = E // NCORES
G = 8
CH = 2048                  # gather stream positions per chunk (per group)
NCH_FULL = 52
POS = CH * NCH_FULL
EPAD = G * POS
NEQ = 391                  # q range (src>>8 max 390)
NEL = NEQ * 32             # scatter_add num_elems: idx = q*32 + lane(j%32)
SNI = 2048                 # edges per scatter_add instruction
TPC = CH // 16

_cache = {}


def _build(nch, passes=1):
    from concourse import bacc, mybir, tile

    nc = bacc.Bacc(None, target_bir_lowering=False)
    f32, bf16, i32, i16 = (
        mybir.dt.float32, mybir.dt.bfloat16, mybir.dt.int32, mybir.dt.int16,
    )

    pos = CH * nch
    dstw = nc.declare_dram_parameter("dstw", [128, pos // 16], i32, isOutput=False)
    srcw = nc.declare_dram_parameter("srcw", [128, G * pos // 16], i32, isOutput=False)
    dst8 = nc.declare_dram_parameter("dst8", [G, pos], i32, isOutput=False)
    src8 = nc.declare_dram_parameter("src8", [G, pos], i32, isOutput=False)
    w8 = nc.declare_dram_parameter("w8", [G, pos], f32, isOutputFalse := False)
    xtab = nc.declare_dram_parameter("xtab", [128, N // 16], f32, isOutput=False)
    iota16c = nc.declare_dram_parameter("iota16c", [128, 1], i32, isOutput=False)
    iota2p0 = nc.declare_dram_parameter("iota2p0", [128, 1], i32, isOutput=False)
    iota2p1 = nc.declare_dram_parameter("iota2p1", [128, 1], i32, isOutput=False)
    iotap8 = nc.declare_dram_parameter("iotap8", [128, 1], i32, isOutput=False)
    iotac2 = nc.declare_dram_parameter("iotac2", [128, SNI // 16], i32, isOutput=False)
    bones = nc.declare_dram_parameter("bones", [128, G], f32, isOutput=False)
    out = nc.declare_dram_parameter("out", [N, 1], f32, isOutput=True)

    msgs_dram = nc.dram_tensor("msgs_dram", [G, pos], f32)
    partial = nc.dram_tensor("partial", [128, 2 * NEQ], f32)
    reduced = nc.dram_tensor("reduced", [128, 2 * NEQ], f32, addr_space="Shared")

    with tile.TileContext(nc) as tc:
        with tc.tile_pool(name="const", bufs=1) as cpool, \
             tc.tile_pool(name="work", bufs=2) as pool, \
             tc.tile_pool(name="acc", bufs=1) as apool, \
             tc.tile_pool(name="sc", bufs=1) as scpool, \
             tc.tile_pool(name="ps16", bufs=2, space="PSUM") as ps16pool:

            xtab_sb = cpool.tile([128, N // 16], f32, tag="xtab")
            nc.sync.dma_start(out=xtab_sb[:], in_=xtab[:])
            i16c_sb = cpool.tile([128, 1], i32, tag="i16c")
            nc.sync.dma_start(out=i16c_sb[:], in_=iota16c[:])
            i2p0_sb = cpool.tile([128, 1], i32, tag="i2p0")
            nc.sync.dma_start(out=i2p0_sb[:], in_=iota2p0[:])
            i2d_sb = cpool.tile([128, 2], i32, tag="i2d")
            nc.sync.dma_start(out=i2d_sb[:, 0:1], in_=iota2p0[:])
            nc.sync.dma_start(out=i2d_sb[:, 1:2], in_=iota2p1[:])
            ip8_sb = cpool.tile([128, 1], i32, tag="ip8")
            nc.sync.dma_start(out=ip8_sb[:], in_=iotap8[:])
            ic2_sb = cpool.tile([128, SNI // 16], i32, tag="ic2")
            nc.sync.dma_start(out=ic2_sb[:], in_=iotac2[:])
            bones_sb = cpool.tile([128, G], f32, tag="bones")
            nc.sync.dma_start(out=bones_sb[:], in_=bones[:])

            acc_bf = apool.tile([128, NEL * 2], bf16, tag="acc_bf")
            acc_f32 = apool.tile([128, NEQ * 2], f32, tag="acc_f32")
            nc.vector.memset(acc_bf[:], 0)
            nc.vector.memset(acc_f32[:], 0)

            for pss in range(passes):
              for ch in range(nch):
                csl = slice(ch * CH, (ch + 1) * CH)
                wsl = slice(ch * TPC, (ch + 1) * TPC)

                # ---- gather ----
                dstw_c = pool.tile([128, TPC], i32, tag="dstw_c")
                nc.sync.dma_start(out=dstw_c[:], in_=dstw[:, wsl])
                idx16 = pool.tile([128, TPC], i16, tag="idx16")
                nc.vector.tensor_scalar(
                    out=dstw_c[:], in0=dstw_c[:], scalar1=4, scalar2=None,
                    op0=mybir.AluOpType.logical_shift_right,
                )
                nc.vector.tensor_copy(out=idx16[:], in_=dstw_c[:])

                cand = pool.tile([128, CH], f32, tag="cand")
                nc.gpsimd.ap_gather(
                    out_ap=cand[:], in_ap=xtab_sb[:], idxs_ap=idx16[:],
                    channels=128, num_elems=N // 16, d=1, num_idxs=CH,
                )

                dstr_c = pool.tile([128, CH], i32, tag="dstr_c")
                nc.sync.dma_start(
                    out=dstr_c[:],
                    in_=dst8[:, csl].unsqueeze(1).to_broadcast([G, 16, CH]),
                )
                nc.vector.tensor_scalar(
                    out=dstr_c[:], in0=dstr_c[:], scalar1=15, scalar2=None,
                    op0=mybir.AluOpType.bitwise_and,
                )
                mask = pool.tile([128, CH], f32, tag="mask")
                nc.vector.tensor_tensor(
                    out=mask[:], in0=dstr_c[:],
                    in1=i16c_sb[:].to_broadcast([128, CH]),
                    op=mybir.AluOpType.is_equal,
                )
                nc.vector.tensor_tensor(
                    out=cand[:], in0=cand[:], in1=mask[:],
                    op=mybir.AluOpType.mult,
                )

                # ---- reduce 16 -> 1 (block-ones matmul) and multiply by w ----
                w8_c = pool.tile([G, CH], f32, tag="w8_c")
                nc.sync.dma_start(out=w8_c[:], in_=w8[:, csl])
                msgs8 = pool.tile([G, CH], f32, tag="msgs8")
                for s in range(CH // 512):
                    ps16 = ps16pool.tile([G, 512], f32, tag="ps16")
                    nc.tensor.matmul(
                        out=ps16[:], lhsT=bones_sb[:],
                        rhs=cand[:, s * 512:(s + 1) * 512],
                        start=True, stop=True,
                    )
                    nc.vector.tensor_tensor(
                        out=msgs8[:, s * 512:(s + 1) * 512], in0=ps16[:],
                        in1=w8_c[:, s * 512:(s + 1) * 512],
                        op=mybir.AluOpType.mult,
                    )
                nc.sync.dma_start(out=msgs_dram[:, csl], in_=msgs8[:])

                # ---- scatter via gpsimd scatter_add, 2 instrs of SNI edges ----
                for hf in range(CH * G // SNI):
                    ihalf = slice(ch * CH + hf * (SNI // G),
                                  ch * CH + (hf + 1) * (SNI // G))
                    # edge j = g*1024 + il ; msgs/src broadcast to 128 partitions
                    msgbc = scpool.tile([128, SNI], f32, tag="msgbc")
                    nc.sync.dma_start(
                        out=msgbc[:].rearrange("p (g i) -> p g i", g=G),
                        in_=msgs_dram[:, ihalf].unsqueeze(0)
                            .to_broadcast([128, G, SNI // G]),
                    )
                    srcbc = scpool.tile([128, SNI], i32, tag="srcbc")
                    nc.sync.dma_start(
                        out=srcbc[:].rearrange("p (g i) -> p g i", g=G),
                        in_=src8[:, ihalf].unsqueeze(0)
                            .to_broadcast([128, G, SNI // G]),
                    )
                    # idx for scatter: srcw slice >> 8 -> int16
                    nhf = CH * G // SNI
                    swl = slice((ch * nhf + hf) * (SNI // 16),
                                (ch * nhf + hf + 1) * (SNI // 16))
                    srcw_c = pool.tile([128, SNI // 16], i32, tag="srcw_c")
                    nc.sync.dma_start(out=srcw_c[:], in_=srcw[:, swl])
                    nc.vector.tensor_scalar(
                        out=srcw_c[:], in0=srcw_c[:], scalar1=8, scalar2=None,
                        op0=mybir.AluOpType.logical_shift_right,
                    )
                    nc.vector.tensor_scalar(
                        out=srcw_c[:], in0=srcw_c[:], scalar1=5, scalar2=None,
                        op0=mybir.AluOpType.logical_shift_left,
                    )
                    nc.vector.tensor_tensor(
                        out=srcw_c[:], in0=srcw_c[:],
                        in1=ip8_sb[:].to_broadcast([128, SNI // 16]),
                        op=mybir.AluOpType.add,
                    )
                    nc.vector.tensor_tensor(
                        out=srcw_c[:], in0=srcw_c[:], in1=ic2_sb[:],
                        op=mybir.AluOpType.add,
                    )
                    sidx = pool.tile([128, SNI // 16], i16, tag="sidx")
                    nc.vector.tensor_copy(out=sidx[:], in_=srcw_c[:])

                    # masked add tile [128, SNI, 2] bf16
                    nc.vector.tensor_scalar(
                        out=srcbc[:], in0=srcbc[:], scalar1=255, scalar2=None,
                        op0=mybir.AluOpType.bitwise_and,
                    )
                    addt = scpool.tile([128, SNI * 2], bf16, tag="addt")
                    m2 = scpool.tile([128, SNI * 2], bf16, tag="m2")
                    nc.vector.tensor_tensor(
                        out=m2[:].rearrange("p (i s) -> p i s", s=2),
                        in0=srcbc[:].unsqueeze(2).to_broadcast([128, SNI, 2]),
                        in1=i2d_sb[:].unsqueeze(1).to_broadcast([128, SNI, 2]),
                        op=mybir.AluOpType.is_equal,
                    )
                    nc.vector.tensor_tensor(
                        out=addt[:].rearrange("p (i s) -> p i s", s=2),
                        in0=m2[:].rearrange("p (i s) -> p i s", s=2),
                        in1=msgbc[:].unsqueeze(2).to_broadcast([128, SNI, 2]),
                        op=mybir.AluOpType.mult,
                    )
                    nc.gpsimd.scatter_add(
                        in_ap=acc_bf[:], idxs_ap=sidx[:], add_ap=addt[:],
                        channels=128, num_elems=NEL, d=2, num_idxs=SNI,
                    )

                # ---- drain bf16 acc (8 lanes) into f32 ----
                for l in range(32):
                    nc.vector.tensor_tensor(
                        out=acc_f32[:].rearrange("p (q s) -> p q s", s=2),
                        in0=acc_f32[:].rearrange("p (q s) -> p q s", s=2),
                        in1=acc_bf[:].rearrange("p (q l s) -> p q l s", l=32, s=2)[:, :, l, :],
                        op=mybir.AluOpType.add,
                    )
                nc.vector.memset(acc_bf[:], 0)

            # ---- combine ----
            nc.sync.dma_start(out=partial[:], in_=acc_f32[:])
            nc.gpsimd.collective_compute(
                "AllReduce",
                mybir.AluOpType.add,
                replica_groups=[list(range(NCORES))],
                ins=[partial[:]],
                outs=[reduced[:]],
            )
            # out[n] = reduced[(n%256)>>1, 2*(n>>8) + (n&1)]
            # out viewed [q, ch, s]: out3[q, c, s] = reduced[c, 2q + s]
            nmain = (N // 256) * 256  # 99840
            with nc.allow_non_contiguous_dma(reason="final relayout"):
                nc.sync.dma_start(
                    out=out[0:nmain, 0].rearrange("(q c s) -> q c s", c=128, s=2),
                    in_=reduced[:, 0:2 * (N // 256)]
                        .rearrange("c (q s) -> q c s", s=2),
                )
                # tail nodes 99840..99999: q = 390, 160 values = 80 ch x 2
                nc.sync.dma_start(
                    out=out[nmain:N, 0].rearrange("(c s) -> c s", s=2),
                    in_=reduced[0:(N - nmain) // 2, 2 * (N // 256):2 * (N // 256) + 2],
                )

    nc.finalize()
    return nc


def _get_nc(nch):
    if nch not in _cache:
        _cache[nch] = _build(nch)
    return _cache[nch]


def _build_timing(nch, passes=2):
    key = (nch, passes)
    if key not in _cache:
        _cache[key] = _build(nch, passes=passes)
    return _cache[key]


def _host_prep(x, edge_index, edge_weight, nch):
    x = np.asarray(x, dtype=np.float32).reshape(N)
    ei = np.asarray(edge_index)
    src = ei[0].astype(np.int32)
    dst = ei[1].astype(np.int32)
    w = np.asarray(edge_weight, dtype=np.float32)

    pos = CH * nch
    epad_core = G * pos
    nuse = min(EC, epad_core)

    xtab = np.tile(np.ascontiguousarray(x.reshape(N // 16, 16).T), (8, 1))
    iota16c = (np.arange(128, dtype=np.int32) % 16).reshape(128, 1)
    iota2p0 = (np.arange(128, dtype=np.int32) * 2).reshape(128, 1)
    iota2p1 = (np.arange(128, dtype=np.int32) * 2 + 1).reshape(128, 1)
    iotap8 = (np.arange(128, dtype=np.int32) % 16).reshape(128, 1)
    iotac2 = np.tile((np.arange(SNI // 16, dtype=np.int32) % 2) * 16, (128, 1))
    bones = np.zeros((128, G), np.float32)
    for g in range(G):
        bones[16 * g:16 * (g + 1), g] = 1.0

    in_maps = []
    for c in range(NCORES):
        s_c = np.zeros(epad_core, np.int32)
        d_c = np.zeros(epad_core, np.int32)
        w_c = np.zeros(epad_core, np.float32)
        lo = c * EC
        s_c[:nuse] = src[lo:lo + nuse]
        d_c[:nuse] = dst[lo:lo + nuse]
        w_c[:nuse] = w[lo:lo + nuse]

        d_g = d_c.reshape(G, pos)
        s_g = s_c.reshape(G, pos)
        w8v = w_c.reshape(G, pos)
        dstwv = np.ascontiguousarray(
            d_g.reshape(G, pos // 16, 16).transpose(0, 2, 1)
        ).reshape(128, pos // 16)
        # srcw: per scatter half (1024 positions), edge j = g*1024+il,
        # wrap16: j at [j%16, j//16]; halves concatenated along free dim,
        # then replicated 8x to 128 partitions.
        nhf = CH * G // SNI
        s_halves = s_g.reshape(G, nhf * nch, SNI // G)      # [g, piece, il]
        s_halves = s_halves.transpose(1, 0, 2).reshape(nhf * nch, SNI)
        srcw16 = np.ascontiguousarray(
            s_halves.reshape(nhf * nch, SNI // 16, 16).transpose(0, 2, 1)
        ).reshape(nhf * nch, 16, SNI // 16)
        srcwv = np.concatenate([srcw16[h] for h in range(nhf * nch)], axis=1)
        srcwv = np.tile(srcwv, (8, 1))

        in_maps.append({
            "dstw": dstwv,
            "srcw": np.ascontiguousarray(srcwv),
            "dst8": np.ascontiguousarray(d_g),
            "src8": np.ascontiguousarray(s_g),
            "w8": np.ascontiguousarray(w8v),
            "xtab": xtab,
            "iota16c": iota16c,
            "iota2p0": iota2p0,
            "iota2p1": iota2p1,
            "iotap8": iotap8,
            "iotac2": iotac2,
            "bones": bones,
        })
    return in_maps


def kernel(x, edge_index, edge_weight, nch=NCH_FULL):
    from concourse.bass_utils import run_bass_kernel_spmd

    nc = _get_nc(nch)
    in_maps = _host_prep(x, edge_index, edge_weight, nch)
    res = run_bass_kernel_spmd(nc, in_maps, list(range(NCORES)))
    out = res.results[0]["out"].astype(np.float32).reshape(N, 1)
    return out

